# revision 21
# baseline (speedup 1.0000x reference)
"""APPNP GNN kernel for 8 Trainium2 NeuronCores (Bass/Tile).

Strategy (pull model, nodes partitioned by destination across cores):
  - MLP h = relu(x@W1.T+b1)@W2.T+b2 is data-parallel over nodes; x is
    pre-transposed on the host so the TensorEngine contracts over features.
  - Propagation state zh = dinv * z (all 100K nodes x 16 classes) is
    SBUF-resident, replicated per core, laid out as [128 partitions =
    (group g, feature f), 12544 node-columns]; group g holds core g's nodes.
    Carrying dinv*z folds the src side of the GCN edge norm into the state.
  - Each step: per dst-chunk a GPSIMD ap_gather pulls zh columns for that
    chunk's (dst-sorted) edges, a DVE prefix scan computes a running cumsum,
    a second ap_gather extracts the cumsum at segment boundaries, and a
    subtract yields per-(group,dst) segment sums.  Self-loop edges are not
    materialized: their term is a contiguous zs slice added via a per-core
    partition mask, which also shrinks the max cell width ~20%.  A one-hot
    matmul on PE reduces the 8 groups, then DVE applies 0.9*dinv^2 and adds
    0.1*dinv*h.
  - An AllGather (bf16) re-replicates zh for the next step.
  - log_softmax fused into the last step via PE transpose + ACT exp/ln.
"""

import sys

sys.path.insert(0, "/opt/trn_rl_repo")

import numpy as np

N_NODES = 100000
K_STEPS = 10
ALPHA = 0.1
F_IN = 512
H1 = 256
C = 16

NCORES = 8
PER_CORE = N_NODES // NCORES      # 12500 real nodes per core
PN = 12544                        # padded nodes per core
NCH = 16                          # dst chunks per step
D = PN // NCH                     # 784 dst nodes per chunk
SPLIT_CH = 12                     # chunks in AllGather part A (prefetched)
EIDX_N = 800                      # (D+1) padded to mult of 16
RCH = 392                         # column chunk for matmuls (f32r wants N>=256)
NBLK = 7                          # 128-wide blocks per dst chunk (6*128+16)

_CACHE = {}
DBG_SKIP_COLL = False
DBG_SKIP_GATHER = False
DBG_SKIP_SCAN = False
DBG_SKIP_EXTRACT = False


def _balance_perm(src, dst):
    """Per-core within-core node relabeling so the (group, chunk) gather
    cells are load-balanced: greedy LPT on the per-group indegree vectors.
    Returns perm_all (new global order, grouped by core): node at new
    global id i is old node perm_all[i]."""
    g = src // PER_CORE
    M = np.bincount(dst * NCORES + g, minlength=N_NODES * NCORES)
    M = M.reshape(N_NODES, NCORES)
    perms = []
    caps = np.full(NCH, D, dtype=np.int64)
    caps[-1] = PER_CORE - (NCH - 1) * D
    for c in range(NCORES):
        vec = M[c * PER_CORE:(c + 1) * PER_CORE]
        order_d = np.argsort(-vec.sum(1), kind="stable")
        V = vec[order_d]
        L = np.zeros((NCH, NCORES), dtype=np.int64)
        counts = np.zeros(NCH, dtype=np.int64)
        assign = np.empty(PER_CORE, dtype=np.int64)
        big = np.int64(1) << 40
        for i in range(PER_CORE):
            m = (L + V[i]).max(axis=1)
            m[counts >= caps] = big
            ch = int(np.argmin(m))
            assign[order_d[i]] = ch
            L[ch] += V[i]
            counts[ch] += 1
        # swap-refinement: drive the max cell toward the per-group lower
        # bound so CW lands on the next-lower multiple of 32.
        L32 = L.astype(np.int64)
        stall = 0
        for _ in range(400):
            mx = L32.max()
            if mx <= 3167 or stall >= 2:
                break
            ch_s, g_s = np.unravel_index(int(np.argmax(L32)), L32.shape)
            in_s = np.flatnonzero(assign == ch_s)
            a_cands = in_s[np.argsort(-vec[in_s, g_s])[:4]]
            best = None
            others = np.flatnonzero(assign != ch_s)
            vb = vec[others]
            Lb = L32[assign[others]]
            for a in a_cands:
                va = vec[a]
                new_s = (L32[ch_s] - va + vb).max(axis=1)
                new_b = (Lb + va - vb).max(axis=1)
                score = np.maximum(new_s, new_b)
                j = int(np.argmin(score))
                if best is None or score[j] < best[0]:
                    best = (int(score[j]), int(a), int(others[j]))
            s, a, b = best
            if s >= mx:
                stall += 1
                continue
            stall = 0
            cb = int(assign[b])
            L32[ch_s] += vec[b] - vec[a]
            L32[cb] += vec[a] - vec[b]
            assign[a], assign[b] = cb, ch_s
        perm = np.concatenate(
            [np.flatnonzero(assign == ch) for ch in range(NCH)])
        perms.append(c * PER_CORE + perm)
    return np.concatenate(perms)


def _prep(x, W1, b1, W2, b2, edge_index):
    import ml_dtypes

    bf16 = ml_dtypes.bfloat16

    src = np.asarray(edge_index[0], dtype=np.int64)
    dst = np.asarray(edge_index[1], dtype=np.int64)
    # degree with self loops (reference: segment_sum of ones over dst+loop)
    deg = (np.bincount(dst, minlength=N_NODES) + 1).astype(np.float64)
    dinv = 1.0 / np.sqrt(deg)
    # self-loop edges are NOT materialized: the self term zh[d] is added
    # straight from the zs slice via a per-core partition mask (selfmask),
    # which also deflates the max cell width (gather/scan volume) ~20%.

    # relabel nodes within each core so dst chunks have balanced per-group
    # indegree sums -> smaller max cell width CW (less gather padding).
    perm_all = _balance_perm(src, dst)
    inv = np.empty(N_NODES, dtype=np.int64)
    inv[perm_all] = np.arange(N_NODES)
    src = inv[src]
    dst = inv[dst]
    dinv = dinv[perm_all]
    x = np.asarray(x, np.float32)[perm_all]

    core_of = dst // PER_CORE
    per_core = []
    chunk_max = np.zeros(NCH, dtype=np.int64)
    for c in range(NCORES):
        sel = core_of == c
        s = src[sel]
        dl = dst[sel] - c * PER_CORE
        g = s // PER_CORE
        sl = s - g * PER_CORE
        ch = dl // D
        cell = g * NCH + ch
        # src-ascending within each (cell, dst) segment: near-sequential
        # SBUF reads measurably speed up the GPSIMD gather pipe
        order = np.lexsort((sl, dl, cell))
        s_sl = sl[order].astype(np.int64)
        s_dl = dl[order]
        s_cell = cell[order]
        counts = np.bincount(s_cell, minlength=8 * NCH)
        np.maximum(chunk_max, counts.reshape(8, NCH).max(axis=0),
                   out=chunk_max)
        per_core.append((s_sl, s_dl, s_cell, counts))

    # chunk capacity (max over cores, groups and chunks): +1 for the
    # reserved zero column, round to mult of 32 (the gather ucode reads
    # index words in uint32 pairs; an odd 16-column count mis-executes).
    # NOTE: per-chunk variable widths mis-execute on HW (verified), so all
    # chunks share the max.
    CW = int(max(((chunk_max.max() + 1 + 31) // 32) * 32, 64))
    percw = (CW,) * NCH
    assert CW <= 32767 and PN <= 32767

    summat = np.zeros((128, C), dtype=bf16)
    summat[np.arange(128), np.arange(128) % 16] = 1.0
    id16 = np.eye(C, dtype=bf16)
    b1m = np.asarray(b1, np.float32).reshape(2, 128).T.copy()    # [128,2]
    b2m = np.asarray(b2, np.float32).reshape(C, 1).copy()        # [16,1]
    w1t = np.ascontiguousarray(np.asarray(W1, np.float32).T).astype(bf16)
    w2t = np.ascontiguousarray(np.asarray(W2, np.float32).T)     # [256,16]

    xf = np.asarray(x, np.float32)
    in_maps = []
    for c in range(NCORES):
        s_sl, s_dl, s_cell, counts = per_core[c]
        V = np.zeros((8 * NCH, CW), dtype=np.int16)
        E = np.zeros((8 * NCH, EIDX_N), dtype=np.int16)
        offs = np.zeros(8 * NCH + 1, dtype=np.int64)
        np.cumsum(counts, out=offs[1:])
        pos = np.arange(s_sl.shape[0]) - offs[s_cell] + 1
        V[s_cell, pos] = s_sl.astype(np.int16)
        for cell in range(8 * NCH):
            lo, hi = offs[cell], offs[cell + 1]
            ch = cell % NCH
            dcnt = np.bincount(s_dl[lo:hi] - ch * D, minlength=D)
            E[cell, 1:D + 1] = np.cumsum(dcnt).astype(np.int16)
            E[cell, D + 1:] = E[cell, D]
        blocks = []
        for ch in range(NCH):
            pc = percw[ch]
            Vch = V.reshape(8, NCH, CW)[:, ch, :pc]
            blocks.append(Vch.reshape(8, pc // 16, 16).transpose(0, 2, 1)
                          .reshape(128, pc // 16))
        gidx = np.ascontiguousarray(np.concatenate(blocks, axis=1))
        eidx = (E.reshape(8, NCH, EIDX_N // 16, 16)
                 .transpose(0, 3, 1, 2).reshape(128, -1).copy())

        dloc = np.zeros(PN, dtype=np.float64)
        dloc[:PER_CORE] = dinv[c * PER_CORE:(c + 1) * PER_CORE]
        # [16, PN] streams: 0.9*dinv^2 (epilogue) and dinv (MLP)
        d92 = np.tile(((1.0 - ALPHA) * dloc * dloc).astype(np.float16), (C, 1))
        dinv16 = np.tile(dloc.astype(np.float16), (C, 1))
        dsafe = np.where(dloc > 0, dloc, 1.0)
        drinv = np.tile((1.0 / dsafe).astype(np.float16), (C, 1))

        xT = np.zeros((F_IN, PN), dtype=bf16)
        xT[:, :PER_CORE] = xf[c * PER_CORE:(c + 1) * PER_CORE].T.astype(bf16)

        selfmask = np.zeros((128, 1), dtype=np.float32)
        selfmask[16 * c:16 * (c + 1), 0] = 1.0

        in_maps.append({
            "xT": xT, "w1t": w1t, "b1m": b1m, "w2t": w2t, "b2m": b2m,
            "gidx": gidx, "eidx": eidx, "d92": d92, "dinv16": dinv16,
            "drinv": drinv, "summat": summat, "id16": id16,
            "selfmask": selfmask,
        })
    return in_maps, percw, perm_all


def _build(percw):
    from concourse import bacc, tile, mybir, library_config

    f32 = mybir.dt.float32
    bf16 = mybir.dt.bfloat16
    f32r = mybir.dt.float32r
    f16 = mybir.dt.float16
    i16 = mybir.dt.int16
    Alu = mybir.AluOpType
    Act = mybir.ActivationFunctionType
    CW = max(percw)
    off16 = [0]
    for pc in percw:
        off16.append(off16[-1] + pc // 16)
    GIDX_COLS = off16[-1]
    EI16 = EIDX_N // 16

    nc = bacc.Bacc(None, target_bir_lowering=False)

    xT = nc.declare_dram_parameter("xT", [F_IN, PN], bf16, False)
    w1t = nc.declare_dram_parameter("w1t", [F_IN, H1], bf16, False)
    b1m = nc.declare_dram_parameter("b1m", [128, 2], f32, False)
    w2t = nc.declare_dram_parameter("w2t", [H1, C], f32, False)
    b2m = nc.declare_dram_parameter("b2m", [C, 1], f32, False)
    gidx_d = nc.declare_dram_parameter("gidx", [128, GIDX_COLS], i16, False)
    eidx_d = nc.declare_dram_parameter("eidx", [128, NCH * EI16], i16, False)
    d92_d = nc.declare_dram_parameter("d92", [C, PN], f16, False)
    dinv16_d = nc.declare_dram_parameter("dinv16", [C, PN], f16, False)
    drinv_d = nc.declare_dram_parameter("drinv", [C, PN], f16, False)
    summat_d = nc.declare_dram_parameter("summat", [128, C], bf16, False)
    id16_d = nc.declare_dram_parameter("id16", [C, C], bf16, False)
    selfmask_d = nc.declare_dram_parameter("selfmask", [128, 1], f32, False)
    out_d = nc.declare_dram_parameter("out", [C, PER_CORE], f32, isOutput=True)

    h01_d = nc.dram_tensor("h01d", [C, PN], bf16)

    with tile.TileContext(nc) as tc:
        nc.gpsimd.load_library(library_config.ap_gather)
        with (
            tc.tile_pool(name="const", bufs=1) as const,
            tc.tile_pool(name="zp", bufs=1) as zp,
            tc.tile_pool(name="work", bufs=2) as work,
            tc.tile_pool(name="scop", bufs=1) as scop,
            tc.tile_pool(name="psum", bufs=2, space="PSUM") as psum,
            tc.tile_pool(name="dram", bufs=2, space="DRAM") as dramp,
        ):
            # ---- constants ----
            gidx_sb = const.tile([128, GIDX_COLS], i16)
            nc.sync.dma_start(gidx_sb[:], gidx_d[:])
            eidx_sb = const.tile([128, NCH * EI16], i16)
            nc.sync.dma_start(eidx_sb[:], eidx_d[:])
            ones16_sb = const.tile([C, 1], f32)
            nc.vector.memset(ones16_sb[:], 1.0)
            ones1_sb = const.tile([1, C], f32)
            nc.vector.memset(ones1_sb[:], 1.0)
            summat_sb = const.tile([128, C], bf16)
            nc.sync.dma_start(summat_sb[:], summat_d[:])
            id16_sb = const.tile([C, C], bf16)
            nc.sync.dma_start(id16_sb[:], id16_d[:])
            zeros_sb = const.tile([128, CW], bf16)
            nc.vector.memset(zeros_sb[:], 0.0)
            selfmask_sb = const.tile([128, 1], f32)
            nc.sync.dma_start(selfmask_sb[:], selfmask_d[:])

            # ---- persistent state ----
            zs = zp.tile([128, PN], f32)       # gather source (f32 copy of zh)
            zt = zp.tile([128, PN], bf16)      # allgather result

            ccins = {}

            # ---- MLP ----
            ccins[0] = dramp.tile([C, PN], bf16, tag="ccin", name="ccin0")
            with tc.tile_pool(name="mlp", bufs=2) as mlp:
                w1t_sb = const.tile([128, 4, H1], bf16)
                for b in range(4):
                    nc.sync.dma_start(w1t_sb[:, b, :],
                                      w1t[128 * b:128 * (b + 1), :])
                b1m_sb = const.tile([128, 2], f32)
                nc.sync.dma_start(b1m_sb[:], b1m[:])
                w2t_sb = const.tile([128, 2, C], f32)
                for m in range(2):
                    nc.sync.dma_start(w2t_sb[:, m, :],
                                      w2t[128 * m:128 * (m + 1), :])
                w2tb = const.tile([128, 2, C], bf16)
                nc.scalar.copy(w2tb[:], w2t_sb[:])
                b2m_sb = const.tile([C, 1], f32)
                nc.sync.dma_start(b2m_sb[:], b2m[:])

                for r in range(PN // RCH):
                    cs = slice(r * RCH, (r + 1) * RCH)
                    xt = mlp.tile([128, 4, RCH], bf16, tag="xt")
                    nc.sync.dma_start(
                        xt[:],
                        xT[:, r * RCH:(r + 1) * RCH]
                        .rearrange("(b p) n -> p b n", b=4))
                    h1 = mlp.tile([128, 2, RCH], bf16, tag="h1")
                    for m in range(2):
                        hp1 = psum.tile([128, RCH], f32, tag="hp1")
                        for b in range(4):
                            nc.tensor.matmul(
                                hp1[:],
                                w1t_sb[:, b, 128 * m:128 * (m + 1)],
                                xt[:, b, :],
                                start=(b == 0), stop=(b == 3))
                        nc.scalar.activation(h1[:, m, :], hp1[:], Act.Relu,
                                             bias=b1m_sb[:, m:m + 1])
                    hp2 = psum.tile([C, RCH], f32, tag="hp2")
                    for m in range(2):
                        nc.tensor.matmul(hp2[:], w2tb[:, m, :], h1[:, m, :],
                                         start=(m == 0), stop=(m == 1))
                    dv16 = mlp.tile([C, RCH], f16, tag="dv16")
                    nc.sync.dma_start(dv16[:], dinv16_d[:, cs])
                    hb = mlp.tile([C, RCH], f32, tag="hb")
                    nc.vector.tensor_scalar(hb[:], hp2[:], b2m_sb[:], None,
                                            Alu.add)
                    z0c = mlp.tile([C, RCH], bf16, tag="z0c")
                    nc.vector.tensor_tensor(z0c[:], hb[:], dv16[:], Alu.mult)
                    nc.sync.dma_start(ccins[0][:, cs], z0c[:])
                    h01c = mlp.tile([C, RCH], bf16, tag="h01c")
                    nc.vector.tensor_scalar(h01c[:], z0c[:], ALPHA, None,
                                            Alu.mult)
                    nc.sync.dma_start(h01_d[:, cs], h01c[:])

            # ---- propagation steps ----
            for k in range(K_STEPS):
                if not DBG_SKIP_COLL:
                    ccout = dramp.tile([NCORES, C, PN], bf16, tag="ccout",
                                       addr_space="Shared")
                    nc.gpsimd.collective_compute(
                        "AllGather", Alu.bypass,
                        ins=[ccins[k][:]], outs=[ccout[:]],
                        replica_groups=[list(range(NCORES))])
                    nc.sync.dma_start(zt[:],
                                      ccout[:].rearrange("r f n -> (r f) n"))
                else:
                    nc.sync.dma_start(zt[0:C, 0:C], ccins[k][:, 0:C])
                nc.scalar.copy(zs[:], zt[:])
                if k < K_STEPS - 1:
                    ccins[k + 1] = dramp.tile([C, PN], bf16, tag="ccin",
                                              name=f"ccin{k+1}")

                def issue_gather(ch):
                    pc = percw[ch]
                    msg = work.tile([128, CW], f32, tag="msg",
                                    name=f"msg{k}_{ch}")
                    if not DBG_SKIP_GATHER:
                        nc.gpsimd.ap_gather(
                            msg[:, 0:pc], zs[:],
                            gidx_sb[:, off16[ch]:off16[ch + 1]],
                            channels=128, num_elems=PN, d=1, num_idxs=pc)
                    else:
                        nc.vector.tensor_copy(msg[:, 0:C], zs[:, 0:C])
                    return msg

                # software pipeline: gather(ch+1) is issued on GPSIMD ahead
                # of extract(ch) so the engine never waits on DVE's scan.
                msgs = {0: issue_gather(0)}
                for ch in range(NCH):
                    dcs = slice(ch * D, (ch + 1) * D)
                    pc = percw[ch]
                    msg = msgs.pop(ch)
                    sco = scop.tile([128, CW], f32, tag="sco")
                    if not DBG_SKIP_SCAN:
                        nc.vector.tensor_tensor_scan(
                            sco[:, 0:pc], zeros_sb[:, 0:pc], msg[:, 0:pc],
                            0.0, Alu.add, Alu.add)
                    else:
                        nc.vector.tensor_copy(sco[:, 0:C], msg[:, 0:C])
                    if ch + 1 < NCH:
                        msgs[ch + 1] = issue_gather(ch + 1)
                    ex = work.tile([128, EIDX_N], f32, tag="ex")
                    if not DBG_SKIP_EXTRACT:
                        nc.gpsimd.ap_gather(
                            ex[:], sco[:, 0:pc],
                            eidx_sb[:, ch * EI16:(ch + 1) * EI16],
                            channels=128, num_elems=pc, d=1, num_idxs=EIDX_N)
                    else:
                        nc.vector.tensor_copy(ex[:], sco[:, 0:EIDX_N])
                    aggd = work.tile([128, D], bf16, tag="aggd")
                    nc.vector.tensor_tensor(aggd[:], ex[:, 1:1 + D],
                                            ex[:, 0:D], Alu.subtract)
                    selft = work.tile([128, D], bf16, tag="selft")
                    nc.vector.tensor_scalar(selft[:], zs[:, dcs],
                                            selfmask_sb[:], None, Alu.mult)
                    agg = work.tile([128, D], bf16, tag="agg")
                    nc.vector.tensor_tensor(agg[:], aggd[:], selft[:],
                                            Alu.add)

                    d92c = work.tile([C, D], f16, tag="d92c")
                    nc.sync.dma_start(d92c[:], d92_d[:, dcs])
                    h01cc = work.tile([C, D], bf16, tag="h01cc")
                    nc.sync.dma_start(h01cc[:], h01_d[:, dcs])
                    znc = work.tile([C, D], bf16, tag="znc")
                    for hh in range(2):
                        hs = slice(hh * RCH, (hh + 1) * RCH)
                        ps = psum.tile([C, RCH], f32, tag="ps")
                        nc.tensor.matmul(ps[:], summat_sb[:],
                                         agg[:, hs], start=True, stop=True)
                        t9 = work.tile([C, RCH], bf16, tag="t9")
                        nc.vector.tensor_tensor(t9[:], ps[:], d92c[:, hs],
                                                Alu.mult)
                        nc.vector.tensor_tensor(znc[:, hs], t9[:],
                                                h01cc[:, hs], Alu.add)
                    if k < K_STEPS - 1:
                        nc.sync.dma_start(ccins[k + 1][:, dcs], znc[:])
                    else:
                        # fused feature-major log_softmax for this chunk:
                        # z = znc / dinv (undo carried dinv scaling);
                        # out = z - ln(sum_f exp(z)) -- z is bounded
                        # (|z| <= max|h| ~ 15) so no max-subtraction needed.
                        W = min(D, PER_CORE - ch * D)
                        if W > 0:
                            drc = work.tile([C, D], f16, tag="drc")
                            nc.sync.dma_start(drc[:], drinv_d[:, dcs])
                            zb = work.tile([C, D], f32, tag="zb")
                            nc.vector.tensor_tensor(zb[:], znc[:], drc[:],
                                                    Alu.mult)
                            exd = work.tile([C, D], f32, tag="exd")
                            nc.scalar.activation(exd[:], zb[:], Act.Exp)
                            lssb = work.tile([1, D], f32, tag="lssb")
                            for hh in range(2):
                                hs = slice(hh * RCH, (hh + 1) * RCH)
                                sp = psum.tile([1, RCH], f32, tag="sp", bufs=1)
                                nc.tensor.matmul(sp[:], ones16_sb[:],
                                                 exd[:, hs],
                                                 start=True, stop=True)
                                nc.scalar.activation(lssb[:, hs], sp[:],
                                                     Act.Ln)
                            res = work.tile([C, D], f32, tag="res")
                            for hh in range(2):
                                hs = slice(hh * RCH, (hh + 1) * RCH)
                                lsp = psum.tile([C, RCH], f32, tag="lsp", bufs=1)
                                nc.tensor.matmul(lsp[:], ones1_sb[:],
                                                 lssb[:, hs], start=True,
                                                 stop=True)
                                nc.vector.tensor_tensor(res[:, hs], zb[:, hs],
                                                        lsp[:], Alu.subtract)
                            nc.sync.dma_start(
                                out_d[:, ch * D:ch * D + W], res[:, 0:W])
    nc.compile()
    return nc


def kernel(x, W1, b1, W2, b2, edge_index):
    from concourse.bass_utils import run_bass_kernel_spmd

    in_maps, percw, perm_all = _prep(x, W1, b1, W2, b2, edge_index)
    key = ("k", percw)
    if key not in _CACHE:
        _CACHE[key] = _build(percw)
    nc = _CACHE[key]
    res = run_bass_kernel_spmd(nc, in_maps, core_ids=list(range(NCORES)))
    outs = [np.asarray(res.results[i]["out"], np.float32).T
            for i in range(NCORES)]
    full = np.empty((N_NODES, C), dtype=np.float32)
    full[perm_all] = np.concatenate(outs, axis=0)
    return full

